# revision 17
# baseline (speedup 1.0000x reference)
"""Trainium2 Bass kernel for nn_AlternativeMVN (8-core SPMD).

Math: the reference collapses algebraically.  c = x@Wc+bc is only used via
u = c@wa, v = c@wb, so u = x@(Wc@wa) + bc.wa (tiny GEMVs).  With
S1 = cumsum(v), S2 = cumsum(v^2), P = (j+1)u + S1, Q = u*S1 + S2 the
identity  (tril(u+v.T) @ tril(u+v.T).T)[i,j] = i>=j ? u_i*P_j + Q_j
: P_i*u_j + Q_i  holds exactly, so the O(n^3) GEMM becomes an O(n^2)
rank-4 masked-matmul tile generation.

Distribution: input-dim (i) sharded 8 ways for the GEMVs + one 64 KB
AllReduce of zT=[u,v,mu_z,var_z] (biases ride the AR as a K=1 matmul row,
host-prescaled by 1/8); output rows sharded 8 ways for tile-gen.  Per-core
row/column masks are host inputs; the rank-dependent local row slices and
the diagonal-block placement use register-offset (partition_id) dynamic
access patterns, so the SPMD graph is identical on all cores and no gpsimd
library ops (and no ~110us mid-kernel ucode reload) are needed.
"""
import sys
import numpy as np

if "/opt/trn_rl_repo" not in sys.path:
    sys.path.insert(0, "/opt/trn_rl_repo")

N, IDIM, CDIM, NCORES = 4096, 2048, 4096, 8
ISH = IDIM // NCORES      # 256 input-dim slice per core
RSH = N // NCORES         # 512 output rows per core
TPM = N // 128            # 32 elems per partition in partition-major form

_BUILT = {}


def _ensure_ntff_hook():
    """Provide antenv.axon_hooks (missing on this image) so trace=True /
    BASS_TRACE paths in run_bass_kernel_spmd work instead of crashing."""
    try:
        import antenv.axon_hooks  # noqa: F401
        return
    except ImportError:
        pass
    import types, contextlib, ctypes, os
    try:
        import antenv
    except ImportError:
        return
    mod = types.ModuleType("antenv.axon_hooks")
    _state = {"hook": None}
    mod.set_axon_ntff_profile_hook = lambda h: _state.__setitem__("hook", h)
    mod.get_axon_ntff_profile_hook = lambda: _state["hook"]
    sys.modules["antenv.axon_hooks"] = mod
    antenv.axon_hooks = mod
    so = "/opt/axon/libaxon_pjrt.so"
    if os.path.exists(so):
        try:
            lib = ctypes.CDLL(so)
            if hasattr(lib, "axon_start_nrt_profile"):
                lib.axon_start_nrt_profile.argtypes = [
                    ctypes.POINTER(ctypes.c_int64), ctypes.c_size_t]
                lib.axon_start_nrt_profile.restype = ctypes.c_int64
                lib.axon_stop_nrt_profile.argtypes = [ctypes.c_char_p]
                lib.axon_stop_nrt_profile.restype = ctypes.c_int64

                @contextlib.contextmanager
                def _hook(output_dir, device_ids):
                    import jax
                    jax.devices()
                    if device_ids:
                        ids = (ctypes.c_int64 * len(device_ids))(*device_ids)
                        rc = lib.axon_start_nrt_profile(ids, len(device_ids))
                    else:
                        rc = lib.axon_start_nrt_profile(None, 0)
                    if rc != 0:
                        raise RuntimeError(f"axon_start_nrt_profile rc={rc}")
                    try:
                        yield
                    finally:
                        n = lib.axon_stop_nrt_profile(str(output_dir).encode())
                        print(f"ntff profile: {n} file(s) -> {output_dir}",
                              file=sys.stderr)

                _state["hook"] = _hook
        except Exception:
            pass


def _build_nc():
    import concourse.bass as bass
    import concourse.bacc as bacc
    import concourse.tile as tile
    import concourse.mybir as mybir

    dt = mybir.dt
    f32 = dt.float32
    bf16 = dt.bfloat16
    AF = mybir.ActivationFunctionType
    OP = mybir.AluOpType
    ds = bass.ds

    nc = bacc.Bacc("TRN2", target_bir_lowering=False, debug=False,
                   num_devices=NCORES)

    # ---- I/O ----  (covT/w2/cbias are host-swizzled to partition-major bf16)
    xT_d = nc.dram_tensor("xT", [ISH, N], bf16, kind="ExternalInput")
    covT_d = nc.dram_tensor("covT", [128, 32 * ISH], bf16, kind="ExternalInput")
    w2_d = nc.dram_tensor("w2", [128, 64], bf16, kind="ExternalInput")
    cb_d = nc.dram_tensor("cbias", [128, 32], bf16, kind="ExternalInput")  # /8
    mvk_d = nc.dram_tensor("mvk", [ISH, 2], bf16, kind="ExternalInput")
    mub_d = nc.dram_tensor("mubias", [1, 1], f32, kind="ExternalInput")   # /8
    vab_d = nc.dram_tensor("varbias", [1, 1], f32, kind="ExternalInput")  # /8
    iota_d = nc.dram_tensor("iota1", [1, N], f32, kind="ExternalInput")
    lst_d = nc.dram_tensor("lstrict", [128, 128], f32, kind="ExternalInput")
    tril_d = nc.dram_tensor("trilS", [128, 128], f32, kind="ExternalInput")
    triu_d = nc.dram_tensor("triuS", [128, 128], f32, kind="ExternalInput")
    diag_d = nc.dram_tensor("diagI", [128, 128], f32, kind="ExternalInput")
    m16_d = nc.dram_tensor("m16", [16, N], bf16, kind="ExternalInput")

    cov_o = nc.dram_tensor("out_cov", [RSH, N], f32, kind="ExternalOutput")
    mu_o = nc.dram_tensor("out_mu", [N, 1], f32, kind="ExternalOutput")

    with tile.TileContext(nc) as tc:
        with tc.tile_pool(name="sbM", bufs=1) as sbM, \
             tc.tile_pool(name="dram", bufs=1, space="DRAM") as dram:

            ones_bf = sbM.tile([1, N], bf16, tag="onesbf")
            nc.vector.memset(ones_bf[:], 1.0)

            cc_in = dram.tile([4, N], f32)
            cc_out = dram.tile([4, N], f32, addr_space="Shared")
            dsc = dram.tile([1, RSH], f32)

            # small prefetches on the gpsimd SWDGE ring (HWDGE rings stay free)
            m16_sb = sbM.tile([16, N], bf16, tag="m16")
            nc.sync.dma_start(m16_sb[:], m16_d[:])
            lst_sb = sbM.tile([128, 128], f32, tag="lst")
            nc.gpsimd.dma_start(lst_sb[:], lst_d[:])
            tril_sb = sbM.tile([128, 128], f32, tag="tril")
            nc.gpsimd.dma_start(tril_sb[:], tril_d[:])
            triu_sb = sbM.tile([128, 128], f32, tag="triu")
            nc.gpsimd.dma_start(triu_sb[:], triu_d[:])
            diag_sb = sbM.tile([128, 128], f32, tag="diag")
            nc.gpsimd.dma_start(diag_sb[:], diag_d[:])
            io_pm = sbM.tile([128, TPM], f32, tag="iopm")
            nc.gpsimd.dma_start(io_pm[:], iota_d[:].rearrange("a (p t) -> (a p) t", p=128))

            # ---------------- phase A: GEMVs (input-dim sharded) -------------
            with tc.tile_pool(name="sbIn", bufs=1) as sbIn:
                w2_sb = sbIn.tile([128, 64], bf16, tag="w2")
                nc.sync.dma_start(w2_sb[:], w2_d[:])
                cb_sb = sbIn.tile([128, 32], bf16, tag="cb")
                nc.sync.dma_start(cb_sb[:], cb_d[:])
                covT_sb = sbIn.tile([128, 32 * ISH], bf16, tag="covT")
                xT_sb = sbIn.tile([128, 2 * N], bf16, tag="xT")
                CQ = 32 * ISH // 4
                for g in range(4):
                    s, h = g // 2, g % 2
                    nc.gpsimd.dma_start(covT_sb[:, CQ * g:CQ * (g + 1)],
                                        covT_d[:, CQ * g:CQ * (g + 1)])
                    nc.scalar.dma_start(
                        xT_sb[:, s * N + h * 2048:s * N + (h + 1) * 2048],
                        xT_d[s * 128:(s + 1) * 128, h * 2048:(h + 1) * 2048])

                with tc.tile_pool(name="psA", bufs=1, space="PSUM") as psA:
                    s_ps = psA.tile([2, 1], f32, tag="s")
                    p_ps0 = psA.tile([128, 2], f32, tag="p0")
                    p_ps1 = psA.tile([128, 2], f32, tag="p1")
                    p_ps = [p_ps0, p_ps1]
                    for jc in range(32):
                        nc.tensor.matmul(s_ps[:], w2_sb[:, 2 * jc:2 * jc + 2],
                                         cb_sb[:, jc:jc + 1],
                                         start=(jc == 0), stop=(jc == 31))
                        for isub in range(2):
                            nc.tensor.matmul(
                                p_ps[isub][:],
                                covT_sb[:, jc * ISH + isub * 128:
                                        jc * ISH + isub * 128 + 128],
                                w2_sb[:, 2 * jc:2 * jc + 2],
                                start=(jc == 0), stop=(jc == 31))

                    W4 = []
                    for isub in range(2):
                        w4 = sbIn.tile([128, 4], bf16, tag=f"w4_{isub}")
                        nc.scalar.copy(w4[:, 0:2], p_ps[isub][:])
                        nc.sync.dma_start(w4[:, 2:4],
                                          mvk_d[isub * 128:(isub + 1) * 128, :])
                        W4.append(w4)

                    # bias column (host-prescaled by 1/8; AR sums 8x)
                    bias4c = sbM.tile([4, 1], f32, tag="bias4c")
                    nc.scalar.copy(bias4c[0:2, :], s_ps[:])
                    nc.sync.dma_start(bias4c[2:3, :], mub_d[:])
                    nc.sync.dma_start(bias4c[3:4, :], vab_d[:])

                # zT = W4.T @ xT (+ bias4 x ones), accumulated over i-subchunks
                with tc.tile_pool(name="psZ", bufs=1, space="PSUM") as psZ:
                    zT_ps = psZ.tile([4, N], f32, tag="z")
                    for isub in range(2):
                        for t in range(8):
                            nc.tensor.matmul(
                                zT_ps[:, 512 * t:512 * (t + 1)],
                                W4[isub][:],
                                xT_sb[:, isub * N + 512 * t:
                                      isub * N + 512 * (t + 1)],
                                start=(isub == 0), stop=(isub == 1))
                    zT_sb = sbIn.tile([4, N], f32, tag="zsb")
                    nc.scalar.activation(zT_sb[:, 0:2048], zT_ps[:, 0:2048],
                                         AF.Identity, bias=bias4c[:], scale=1.0)
                    nc.vector.tensor_scalar(zT_sb[:, 2048:4096],
                                            zT_ps[:, 2048:4096],
                                            bias4c[:], None, op0=OP.add)
                    nc.sync.dma_start(cc_in[:], zT_sb[:])

            # ---------------- AllReduce ----------------
            nc.gpsimd.collective_compute(
                "AllReduce", OP.add,
                replica_groups=[list(range(NCORES))],
                ins=[cc_in.opt()], outs=[cc_out.opt()])

            # mu output = AR row 2, straight DRAM->DRAM
            nc.scalar.dma_start(mu_o[:], cc_out[2:3, :])

            with tc.tile_pool(name="sbP", bufs=1) as sbP:
                # ---------------- phase B: P/Q/d vectors ----------------
                z_pm = sbM.tile([128, 4 * TPM], f32, tag="zpm")
                nc.sync.dma_start(
                    z_pm[:].rearrange("p (r t) -> p r t", t=TPM),
                    cc_out[:].rearrange("r (p t) -> p r t", p=128))
                u_pm = z_pm[:, 0 * TPM:1 * TPM]
                v_pm = z_pm[:, 1 * TPM:2 * TPM]
                vz_pm = z_pm[:, 3 * TPM:4 * TPM]

                vsq = sbM.tile([128, TPM], f32, tag="vsq")
                nc.vector.tensor_mul(vsq[:], v_pm[:], v_pm[:])
                s1c = sbM.tile([128, TPM], f32, tag="s1c")
                nc.vector.tensor_tensor_scan(s1c[:], v_pm[:], v_pm[:], 0.0,
                                             op0=OP.add, op1=OP.bypass)
                s2c = sbM.tile([128, TPM], f32, tag="s2c")
                nc.vector.tensor_tensor_scan(s2c[:], vsq[:], vsq[:], 0.0,
                                             op0=OP.add, op1=OP.bypass)
                tot = sbM.tile([128, 2], f32, tag="tot")
                nc.vector.tensor_copy(tot[:, 0:1], s1c[:, TPM - 1:TPM])
                nc.vector.tensor_copy(tot[:, 1:2], s2c[:, TPM - 1:TPM])
                with tc.tile_pool(name="psB", bufs=1, space="PSUM") as psB:
                    offs_ps = psB.tile([128, 2], f32, tag="offs")
                    nc.tensor.matmul(offs_ps[:], lst_sb[:], tot[:], start=True, stop=True)
                    offs_sb = sbM.tile([128, 2], f32, tag="offsb")
                    nc.scalar.copy(offs_sb[:], offs_ps[:])
                S1 = sbM.tile([128, TPM], f32, tag="S1")
                nc.vector.tensor_scalar(S1[:], s1c[:], offs_sb[:, 0:1], None, op0=OP.add)
                S2 = sbM.tile([128, TPM], f32, tag="S2")
                nc.vector.tensor_scalar(S2[:], s2c[:], offs_sb[:, 1:2], None, op0=OP.add)

                # P/Q in bf16 (the K4 matmuls consume bf16 anyway)
                P_pm = sbM.tile([128, TPM], f32, tag="Ppm")
                nc.vector.tensor_mul(P_pm[:], io_pm[:], u_pm[:])
                nc.vector.tensor_add(P_pm[:], P_pm[:], S1[:])
                Q_pm = sbM.tile([128, TPM], f32, tag="Qpm")
                nc.vector.tensor_mul(Q_pm[:], u_pm[:], S1[:])
                nc.vector.tensor_add(Q_pm[:], Q_pm[:], S2[:])
                P_bf = sbM.tile([128, TPM], bf16, tag="Pbf")
                nc.vector.tensor_copy(P_bf[:], P_pm[:])
                Q_bf = sbM.tile([128, TPM], bf16, tag="Qbf")
                nc.vector.tensor_copy(Q_bf[:], Q_pm[:])
                u_bf = sbM.tile([128, TPM], bf16, tag="ubf")
                nc.vector.tensor_copy(u_bf[:], u_pm[:])

                # d = ln(exp(vz) + 1) + 1e-8   (varbias already inside vz)
                vzb = sbM.tile([128, TPM], f32, tag="vzb")
                nc.scalar.activation(vzb[:], vz_pm[:], AF.Exp)
                d_pm = sbM.tile([128, TPM], f32, tag="dpm")
                nc.scalar.activation(d_pm[:], vzb[:], AF.Ln, bias=1.0, scale=1.0)
                nc.vector.tensor_scalar(d_pm[:], d_pm[:], 1e-8, None, op0=OP.add)

                # PQu1 = rows (P, Q, u, 1) bf16 via pm->row transposes
                PQu1 = sbP.tile([4, N], bf16, tag="PQu1")
                nc.sync.dma_start(PQu1[0:1, :], P_bf[:])
                nc.scalar.dma_start(PQu1[1:2, :], Q_bf[:])
                nc.sync.dma_start(PQu1[2:3, :], u_bf[:])
                nc.scalar.dma_start(PQu1[3:4, :], ones_bf[0:1, :])
                d_row = sbP.tile([1, N], f32, tag="drow")
                nc.sync.dma_start(d_row[0:1, :], d_pm[:])

                # local row slices via partition_id dynamic offsets
                rankv = nc.vector.partition_id()
                off_loc = rankv * RSH
                loc4 = sbP.tile([4, RSH], bf16, tag="loc4")
                nc.vector.tensor_copy(loc4[:], PQu1[:, ds(off_loc, RSH)])
                d_loc = sbP.tile([1, RSH], f32, tag="dloc")
                nc.vector.tensor_copy(d_loc[:], d_row[0:1, ds(off_loc, RSH)])
                lhsT4 = sbP.tile([4, RSH], bf16, tag="lhsT4")   # (u, 1, P, Q)
                nc.sync.dma_start(lhsT4[0:2, :], loc4[2:4, :])
                nc.sync.dma_start(lhsT4[2:4, :], loc4[0:2, :])

                nc.scalar.dma_start(dsc[:], d_loc[0:1, :])
                # A16 = PQu1 x4; rhs16 = A16 * m16 (rows 4b+3 = ones*up_b)
                A16 = sbP.tile([16, N], bf16, tag="A16")
                for b in range(4):
                    eng = nc.sync if b % 2 == 0 else nc.scalar
                    eng.dma_start(A16[4 * b:4 * b + 4, :], PQu1[:])
                rhs16 = sbP.tile([16, N], bf16, tag="rhs16")
                nc.vector.tensor_mul(rhs16[:], A16[:], m16_sb[:])
                r4s = []
                for b in range(4):
                    r4 = sbP.tile([4, N], bf16, tag=f"r4_{b}")
                    eng = nc.sync if b % 2 == 0 else nc.scalar
                    eng.dma_start(r4[:], rhs16[4 * b:4 * b + 4, :])
                    r4s.append(r4)
                dsegs = []
                for b in range(4):
                    dsg = sbP.tile([128, 1], f32, tag=f"dseg{b}")
                    nc.sync.dma_start(dsg[:], dsc[0:1, 128 * b:128 * (b + 1)])
                    dsegs.append(dsg)

                # ---------------- phase C: tile generation ----------------
                with tc.tile_pool(name="psC", bufs=1, space="PSUM") as psC, \
                     tc.tile_pool(name="sbC", bufs=1) as sbC:

                    def ps_copy(dst, src, use_vec):
                        if use_vec:
                            nc.vector.tensor_copy(dst, src)
                        else:
                            nc.scalar.copy(dst, src)

                    # PE warm-up during the post-AR DVE chain (HAM K=8/8)
                    wrm = sbC.tile([128, 512], bf16, tag="wrm")
                    nc.vector.memset(wrm[:], 0.125)
                    dum_ps = psC.tile([32, 512], f32, tag="dum")
                    for _ in range(20):
                        nc.tensor.matmul(dum_ps[:], u_bf[:, 0:32],
                                         wrm[:], start=True, stop=True)

                    for b in range(4):
                        r4 = r4s[b]
                        lT = lhsT4[:, 128 * b:128 * (b + 1)]
                        stg = sbC.tile([128, N], f32, tag="stg", bufs=2)
                        for t in range(8):
                            mm_ps = psC.tile([128, 512], f32, tag="chunk", bufs=3)
                            nc.tensor.matmul(mm_ps[:], lT,
                                             r4[:, 512 * t:512 * (t + 1)],
                                             start=True, stop=True)
                            ps_copy(stg[:, 512 * t:512 * (t + 1)], mm_ps[:],
                                    use_vec=(t % 2 == 1))
                        # diagonal block overwrites the masked hole
                        Tl_ps = psC.tile([128, 128], f32, tag="Tl", bufs=2)
                        nc.tensor.matmul(Tl_ps[:], lhsT4[0:2, 128 * b:128 * (b + 1)],
                                         loc4[0:2, 128 * b:128 * (b + 1)],
                                         start=True, stop=True)
                        Tu_ps = psC.tile([128, 128], f32, tag="Tu", bufs=2)
                        nc.tensor.matmul(Tu_ps[:], loc4[0:2, 128 * b:128 * (b + 1)],
                                         lhsT4[0:2, 128 * b:128 * (b + 1)],
                                         start=True, stop=True)
                        t1 = sbC.tile([128, 128], f32, tag="t1", bufs=2)
                        nc.vector.tensor_mul(t1[:], Tl_ps[:], tril_sb[:])
                        t2 = sbC.tile([128, 128], f32, tag="t2", bufs=2)
                        nc.vector.tensor_mul(t2[:], Tu_ps[:], triu_sb[:])
                        t3 = sbC.tile([128, 128], f32, tag="t3", bufs=2)
                        nc.vector.tensor_scalar(t3[:], diag_sb[:], dsegs[b][:],
                                                None, op0=OP.mult)
                        nc.vector.tensor_add(t1[:], t1[:], t2[:])
                        nc.vector.tensor_add(stg[:, ds(off_loc + 128 * b, 128)],
                                             t1[:], t3[:])
                        nc.sync.dma_start(
                            cov_o[128 * b:128 * (b + 1), 0:2048], stg[:, 0:2048])
                        nc.scalar.dma_start(
                            cov_o[128 * b:128 * (b + 1), 2048:4096],
                            stg[:, 2048:4096])

    nc.compile()
    return nc


def _host_shards(x, mu_kernel, mu_bias, cov_kernel, cov_bias, var_kernel,
                 var_bias, rho_kernel):
    import ml_dtypes
    f = np.float32
    bf = ml_dtypes.bfloat16
    covT = cov_kernel.T.astype(f, copy=False)            # [4096, 2048]
    xT = x.T.astype(f, copy=False)
    w2 = np.stack([rho_kernel[:CDIM], rho_kernel[CDIM:]], axis=1).astype(f)
    # partition-major packs: [j*128+p, c] -> [p, j*C + c]
    w2_pk = np.ascontiguousarray(
        w2.reshape(32, 128, 2).transpose(1, 0, 2).reshape(128, 64).astype(bf))
    cb_pk = np.ascontiguousarray(
        (cov_bias.reshape(32, 128, 1).astype(f) / NCORES)
        .transpose(1, 0, 2).reshape(128, 32).astype(bf))
    mvk = np.ascontiguousarray(
        np.concatenate([mu_kernel.reshape(IDIM, 1), var_kernel.reshape(IDIM, 1)],
                       axis=1).astype(bf))
    iota1 = (np.arange(N, dtype=f) + 1.0).reshape(1, N)
    lstrict = np.ascontiguousarray(np.triu(np.ones((128, 128), f), 1))
    trilS = np.ascontiguousarray(np.tril(np.ones((128, 128), f), -1))
    triuS = np.ascontiguousarray(np.triu(np.ones((128, 128), f), 1))
    diagI = np.ascontiguousarray(np.eye(128, dtype=f))
    mub = np.ascontiguousarray(mu_bias.reshape(1, 1).astype(f) / NCORES)
    vab = np.ascontiguousarray(var_bias.reshape(1, 1).astype(f) / NCORES)

    l = np.arange(N)
    in_maps = []
    for k in range(NCORES):
        m16 = np.zeros((16, N), f)
        for b in range(4):
            r0 = RSH * k + 128 * b
            lo = (l < r0).astype(f)
            up = (l >= r0 + 128).astype(f)
            m16[4 * b + 0] = lo
            m16[4 * b + 1] = lo
            m16[4 * b + 2] = up
            m16[4 * b + 3] = up
        covc = covT[:, ISH * k:ISH * (k + 1)]            # [4096, 256]
        cov_pk = np.ascontiguousarray(
            covc.reshape(32, 128, ISH).transpose(1, 0, 2)
            .reshape(128, 32 * ISH).astype(bf))
        in_maps.append({
            "xT": np.ascontiguousarray(xT[ISH * k:ISH * (k + 1), :].astype(bf)),
            "covT": cov_pk,
            "w2": w2_pk, "cbias": cb_pk,
            "mvk": np.ascontiguousarray(mvk[ISH * k:ISH * (k + 1), :]),
            "mubias": mub, "varbias": vab,
            "iota1": iota1, "lstrict": lstrict, "trilS": trilS,
            "triuS": triuS, "diagI": diagI,
            "m16": m16.astype(bf),
        })
    return in_maps


def kernel(x, mu_kernel, mu_bias, cov_kernel, cov_bias, var_kernel, var_bias,
           rho_kernel, _trace=False):
    _ensure_ntff_hook()
    from concourse import bass_utils

    in_maps = _host_shards(np.asarray(x), np.asarray(mu_kernel),
                           np.asarray(mu_bias), np.asarray(cov_kernel),
                           np.asarray(cov_bias), np.asarray(var_kernel),
                           np.asarray(var_bias), np.asarray(rho_kernel))
    if "nc" not in _BUILT:
        _BUILT["nc"] = _build_nc()
    nc = _BUILT["nc"]

    res = bass_utils.run_bass_kernel_spmd(nc, in_maps, core_ids=list(range(NCORES)),
                                          trace=_trace)
    outs = res.results
    out_cov = np.concatenate([outs[k]["out_cov"] for k in range(NCORES)],
                             axis=0).astype(np.float32, copy=False)
    out_mu = outs[0]["out_mu"].astype(np.float32, copy=False)
    if _trace:
        return (out_mu, out_cov), res
    return out_mu, out_cov


# revision 19
# speedup vs baseline: 1.6253x; 1.6253x over previous
"""Trainium2 Bass kernel for nn_AlternativeMVN (8-core SPMD).

Math: the reference collapses algebraically.  c = x@Wc+bc is only used via
u = c@wa, v = c@wb, so u = x@(Wc@wa) + bc.wa (tiny GEMVs).  With
S1 = cumsum(v), S2 = cumsum(v^2), P = (j+1)u + S1, Q = u*S1 + S2 the
identity  (tril(u+v.T) @ tril(u+v.T).T)[i,j] = i>=j ? u_i*P_j + Q_j
: P_i*u_j + Q_i  holds exactly, so the O(n^3) GEMM becomes an O(n^2)
rank-4 masked-matmul tile generation.

Distribution: input-dim (i) sharded 8 ways for the GEMVs + one 64 KB
AllReduce of zT=[u,v,mu_z,var_z] (biases ride the AR as a K=1 matmul row,
host-prescaled by 1/8); output rows sharded 8 ways for tile-gen.  Per-core
row/column masks are host inputs; the rank-dependent local row slices and
the diagonal-block placement use register-offset (partition_id) dynamic
access patterns, so the SPMD graph is identical on all cores and no gpsimd
library ops (and no ~110us mid-kernel ucode reload) are needed.
"""
import sys
import numpy as np

if "/opt/trn_rl_repo" not in sys.path:
    sys.path.insert(0, "/opt/trn_rl_repo")

N, IDIM, CDIM, NCORES = 4096, 2048, 4096, 8
ISH = IDIM // NCORES      # 256 input-dim slice per core
RSH = N // NCORES         # 512 output rows per core
TPM = N // 128            # 32 elems per partition in partition-major form

_BUILT = {}


def _ensure_ntff_hook():
    """Provide antenv.axon_hooks (missing on this image) so trace=True /
    BASS_TRACE paths in run_bass_kernel_spmd work instead of crashing."""
    try:
        import antenv.axon_hooks  # noqa: F401
        return
    except ImportError:
        pass
    import types, contextlib, ctypes, os
    try:
        import antenv
    except ImportError:
        return
    mod = types.ModuleType("antenv.axon_hooks")
    _state = {"hook": None}
    mod.set_axon_ntff_profile_hook = lambda h: _state.__setitem__("hook", h)
    mod.get_axon_ntff_profile_hook = lambda: _state["hook"]
    sys.modules["antenv.axon_hooks"] = mod
    antenv.axon_hooks = mod
    so = "/opt/axon/libaxon_pjrt.so"
    if os.path.exists(so):
        try:
            lib = ctypes.CDLL(so)
            if hasattr(lib, "axon_start_nrt_profile"):
                lib.axon_start_nrt_profile.argtypes = [
                    ctypes.POINTER(ctypes.c_int64), ctypes.c_size_t]
                lib.axon_start_nrt_profile.restype = ctypes.c_int64
                lib.axon_stop_nrt_profile.argtypes = [ctypes.c_char_p]
                lib.axon_stop_nrt_profile.restype = ctypes.c_int64

                @contextlib.contextmanager
                def _hook(output_dir, device_ids):
                    import jax
                    jax.devices()
                    if device_ids:
                        ids = (ctypes.c_int64 * len(device_ids))(*device_ids)
                        rc = lib.axon_start_nrt_profile(ids, len(device_ids))
                    else:
                        rc = lib.axon_start_nrt_profile(None, 0)
                    if rc != 0:
                        raise RuntimeError(f"axon_start_nrt_profile rc={rc}")
                    try:
                        yield
                    finally:
                        n = lib.axon_stop_nrt_profile(str(output_dir).encode())
                        print(f"ntff profile: {n} file(s) -> {output_dir}",
                              file=sys.stderr)

                _state["hook"] = _hook
        except Exception:
            pass


def _build_nc():
    import concourse.bass as bass
    import concourse.bacc as bacc
    import concourse.tile as tile
    import concourse.mybir as mybir

    dt = mybir.dt
    f32 = dt.float32
    bf16 = dt.bfloat16
    AF = mybir.ActivationFunctionType
    OP = mybir.AluOpType
    ds = bass.ds

    nc = bacc.Bacc("TRN2", target_bir_lowering=False, debug=False,
                   num_devices=NCORES)

    # ---- I/O ----  (covT/w2/cbias are host-swizzled to partition-major bf16)
    xT_d = nc.dram_tensor("xT", [ISH, N], bf16, kind="ExternalInput")
    covT_d = nc.dram_tensor("covT", [128, 32 * ISH], bf16, kind="ExternalInput")
    w2_d = nc.dram_tensor("w2", [128, 64], bf16, kind="ExternalInput")
    cb_d = nc.dram_tensor("cbias", [128, 32], bf16, kind="ExternalInput")  # /8
    mvk_d = nc.dram_tensor("mvk", [ISH, 2], bf16, kind="ExternalInput")
    mub_d = nc.dram_tensor("mubias", [1, 1], f32, kind="ExternalInput")   # /8
    vab_d = nc.dram_tensor("varbias", [1, 1], f32, kind="ExternalInput")  # /8
    iota_d = nc.dram_tensor("iota1", [1, N], f32, kind="ExternalInput")
    lst_d = nc.dram_tensor("lstrict", [128, 128], f32, kind="ExternalInput")
    tril_d = nc.dram_tensor("trilS", [128, 128], f32, kind="ExternalInput")
    triu_d = nc.dram_tensor("triuS", [128, 128], f32, kind="ExternalInput")
    diag_d = nc.dram_tensor("diagI", [128, 128], f32, kind="ExternalInput")
    m16_d = nc.dram_tensor("m16", [16, N], bf16, kind="ExternalInput")

    cov_o = nc.dram_tensor("out_cov", [RSH, N], f32, kind="ExternalOutput")
    mu_o = nc.dram_tensor("out_mu", [N, 1], f32, kind="ExternalOutput")

    with tile.TileContext(nc) as tc:
        with tc.tile_pool(name="sbM", bufs=1) as sbM, \
             tc.tile_pool(name="dram", bufs=1, space="DRAM") as dram:

            ones_bf = sbM.tile([1, N], bf16, tag="onesbf")
            nc.vector.memset(ones_bf[:], 1.0)

            cc_in = dram.tile([4, N], f32)
            cc_out = dram.tile([4, N], f32, addr_space="Shared")
            dsc = dram.tile([1, RSH], f32)

            # small prefetches on the gpsimd SWDGE ring (HWDGE rings stay free)
            m16_sb = sbM.tile([16, N], bf16, tag="m16")
            nc.sync.dma_start(m16_sb[:], m16_d[:])
            lst_sb = sbM.tile([128, 128], f32, tag="lst")
            nc.gpsimd.dma_start(lst_sb[:], lst_d[:])
            tril_sb = sbM.tile([128, 128], f32, tag="tril")
            nc.gpsimd.dma_start(tril_sb[:], tril_d[:])
            triu_sb = sbM.tile([128, 128], f32, tag="triu")
            nc.gpsimd.dma_start(triu_sb[:], triu_d[:])
            diag_sb = sbM.tile([128, 128], f32, tag="diag")
            nc.gpsimd.dma_start(diag_sb[:], diag_d[:])
            io_pm = sbM.tile([128, TPM], f32, tag="iopm")
            nc.gpsimd.dma_start(io_pm[:], iota_d[:].rearrange("a (p t) -> (a p) t", p=128))

            # ---------------- phase A: GEMVs (input-dim sharded) -------------
            with tc.tile_pool(name="sbIn", bufs=1) as sbIn:
                w2_sb = sbIn.tile([128, 64], bf16, tag="w2")
                nc.sync.dma_start(w2_sb[:], w2_d[:])
                cb_sb = sbIn.tile([128, 32], bf16, tag="cb")
                nc.sync.dma_start(cb_sb[:], cb_d[:])
                covT_sb = sbIn.tile([128, 32 * ISH], bf16, tag="covT")
                xT_sb = sbIn.tile([128, 2 * N], bf16, tag="xT")
                CQ = 32 * ISH // 4
                for g in range(4):
                    s, h = g // 2, g % 2
                    nc.gpsimd.dma_start(covT_sb[:, CQ * g:CQ * (g + 1)],
                                        covT_d[:, CQ * g:CQ * (g + 1)])
                    nc.scalar.dma_start(
                        xT_sb[:, s * N + h * 2048:s * N + (h + 1) * 2048],
                        xT_d[s * 128:(s + 1) * 128, h * 2048:(h + 1) * 2048])

                with tc.tile_pool(name="psA", bufs=1, space="PSUM") as psA:
                    s_ps = psA.tile([2, 1], f32, tag="s")
                    p_ps0 = psA.tile([128, 2], f32, tag="p0")
                    p_ps1 = psA.tile([128, 2], f32, tag="p1")
                    p_ps = [p_ps0, p_ps1]
                    for jc in range(32):
                        nc.tensor.matmul(s_ps[:], w2_sb[:, 2 * jc:2 * jc + 2],
                                         cb_sb[:, jc:jc + 1],
                                         start=(jc == 0), stop=(jc == 31))
                        for isub in range(2):
                            nc.tensor.matmul(
                                p_ps[isub][:],
                                covT_sb[:, jc * ISH + isub * 128:
                                        jc * ISH + isub * 128 + 128],
                                w2_sb[:, 2 * jc:2 * jc + 2],
                                start=(jc == 0), stop=(jc == 31))

                    W4 = []
                    for isub in range(2):
                        w4 = sbIn.tile([128, 4], bf16, tag=f"w4_{isub}")
                        nc.scalar.copy(w4[:, 0:2], p_ps[isub][:])
                        nc.sync.dma_start(w4[:, 2:4],
                                          mvk_d[isub * 128:(isub + 1) * 128, :])
                        W4.append(w4)

                    # bias column (host-prescaled by 1/8; AR sums 8x)
                    bias4c = sbM.tile([4, 1], f32, tag="bias4c")
                    nc.scalar.copy(bias4c[0:2, :], s_ps[:])
                    nc.sync.dma_start(bias4c[2:3, :], mub_d[:])
                    nc.sync.dma_start(bias4c[3:4, :], vab_d[:])

                # zT = W4.T @ xT (+ bias4 x ones), accumulated over i-subchunks
                with tc.tile_pool(name="psZ", bufs=1, space="PSUM") as psZ:
                    zT_ps = psZ.tile([4, N], f32, tag="z")
                    for isub in range(2):
                        for t in range(8):
                            nc.tensor.matmul(
                                zT_ps[:, 512 * t:512 * (t + 1)],
                                W4[isub][:],
                                xT_sb[:, isub * N + 512 * t:
                                      isub * N + 512 * (t + 1)],
                                start=(isub == 0), stop=(isub == 1))
                    zT_sb = sbIn.tile([4, N], f32, tag="zsb")
                    nc.scalar.activation(zT_sb[:, 0:2048], zT_ps[:, 0:2048],
                                         AF.Identity, bias=bias4c[:], scale=1.0)
                    nc.vector.tensor_scalar(zT_sb[:, 2048:4096],
                                            zT_ps[:, 2048:4096],
                                            bias4c[:], None, op0=OP.add)
                    nc.sync.dma_start(cc_in[:], zT_sb[:])

            # ---------------- AllReduce ----------------
            nc.gpsimd.collective_compute(
                "AllReduce", OP.add,
                replica_groups=[list(range(NCORES))],
                ins=[cc_in.opt()], outs=[cc_out.opt()])

            # mu output = AR row 2, straight DRAM->DRAM
            nc.scalar.dma_start(mu_o[:], cc_out[2:3, :])

            with tc.tile_pool(name="sbP", bufs=1) as sbP:
                # ---------------- phase B: P/Q/d vectors ----------------
                z_pm = sbM.tile([128, 4 * TPM], f32, tag="zpm")
                nc.sync.dma_start(
                    z_pm[:].rearrange("p (r t) -> p r t", t=TPM),
                    cc_out[:].rearrange("r (p t) -> p r t", p=128))
                u_pm = z_pm[:, 0 * TPM:1 * TPM]
                v_pm = z_pm[:, 1 * TPM:2 * TPM]
                vz_pm = z_pm[:, 3 * TPM:4 * TPM]

                vsq = sbM.tile([128, TPM], f32, tag="vsq")
                nc.vector.tensor_mul(vsq[:], v_pm[:], v_pm[:])
                s1c = sbM.tile([128, TPM], f32, tag="s1c")
                nc.vector.tensor_tensor_scan(s1c[:], v_pm[:], v_pm[:], 0.0,
                                             op0=OP.add, op1=OP.bypass)
                s2c = sbM.tile([128, TPM], f32, tag="s2c")
                nc.vector.tensor_tensor_scan(s2c[:], vsq[:], vsq[:], 0.0,
                                             op0=OP.add, op1=OP.bypass)
                tot = sbM.tile([128, 2], f32, tag="tot")
                nc.vector.tensor_copy(tot[:, 0:1], s1c[:, TPM - 1:TPM])
                nc.vector.tensor_copy(tot[:, 1:2], s2c[:, TPM - 1:TPM])
                with tc.tile_pool(name="psB", bufs=1, space="PSUM") as psB:
                    offs_ps = psB.tile([128, 2], f32, tag="offs")
                    nc.tensor.matmul(offs_ps[:], lst_sb[:], tot[:], start=True, stop=True)
                    offs_sb = sbM.tile([128, 2], f32, tag="offsb")
                    nc.scalar.copy(offs_sb[:], offs_ps[:])
                S1 = sbM.tile([128, TPM], f32, tag="S1")
                nc.vector.tensor_scalar(S1[:], s1c[:], offs_sb[:, 0:1], None, op0=OP.add)
                S2 = sbM.tile([128, TPM], f32, tag="S2")
                nc.vector.tensor_scalar(S2[:], s2c[:], offs_sb[:, 1:2], None, op0=OP.add)

                # P/Q in bf16 (the K4 matmuls consume bf16 anyway)
                P_pm = sbM.tile([128, TPM], f32, tag="Ppm")
                nc.vector.tensor_mul(P_pm[:], io_pm[:], u_pm[:])
                nc.vector.tensor_add(P_pm[:], P_pm[:], S1[:])
                Q_pm = sbM.tile([128, TPM], f32, tag="Qpm")
                nc.vector.tensor_mul(Q_pm[:], u_pm[:], S1[:])
                nc.vector.tensor_add(Q_pm[:], Q_pm[:], S2[:])
                P_bf = sbM.tile([128, TPM], bf16, tag="Pbf")
                nc.vector.tensor_copy(P_bf[:], P_pm[:])
                Q_bf = sbM.tile([128, TPM], bf16, tag="Qbf")
                nc.vector.tensor_copy(Q_bf[:], Q_pm[:])
                u_bf = sbM.tile([128, TPM], bf16, tag="ubf")
                nc.vector.tensor_copy(u_bf[:], u_pm[:])

                # d = ln(exp(vz) + 1) + 1e-8   (varbias already inside vz)
                vzb = sbM.tile([128, TPM], f32, tag="vzb")
                nc.scalar.activation(vzb[:], vz_pm[:], AF.Exp)
                d_pm = sbM.tile([128, TPM], f32, tag="dpm")
                nc.scalar.activation(d_pm[:], vzb[:], AF.Ln, bias=1.0, scale=1.0)
                nc.vector.tensor_scalar(d_pm[:], d_pm[:], 1e-8, None, op0=OP.add)

                # PQu1 = rows (P, Q, u, 1) bf16 via pm->row transposes
                PQu1 = sbP.tile([4, N], bf16, tag="PQu1")
                nc.sync.dma_start(PQu1[0:1, :], P_bf[:])
                nc.scalar.dma_start(PQu1[1:2, :], Q_bf[:])
                nc.sync.dma_start(PQu1[2:3, :], u_bf[:])
                nc.scalar.dma_start(PQu1[3:4, :], ones_bf[0:1, :])
                d_row = sbP.tile([1, N], f32, tag="drow")
                nc.sync.dma_start(d_row[0:1, :], d_pm[:])

                # local row slices via partition_id dynamic offsets
                rankv = nc.vector.partition_id()
                off_loc = rankv * RSH
                loc4 = sbP.tile([4, RSH], bf16, tag="loc4")
                nc.vector.tensor_copy(loc4[:], PQu1[:, ds(off_loc, RSH)])
                d_loc = sbP.tile([1, RSH], f32, tag="dloc")
                nc.vector.tensor_copy(d_loc[:], d_row[0:1, ds(off_loc, RSH)])
                lhsT4 = sbP.tile([4, RSH], bf16, tag="lhsT4")   # (u, 1, P, Q)
                nc.sync.dma_start(lhsT4[0:2, :], loc4[2:4, :])
                nc.sync.dma_start(lhsT4[2:4, :], loc4[0:2, :])

                nc.scalar.dma_start(dsc[:], d_loc[0:1, :])
                # A16 = PQu1 x4; rhs16 = A16 * m16 (rows 4b+3 = ones*up_b)
                A16 = sbP.tile([16, N], bf16, tag="A16")
                for b in range(4):
                    eng = nc.sync if b % 2 == 0 else nc.scalar
                    eng.dma_start(A16[4 * b:4 * b + 4, :], PQu1[:])
                rhs16 = sbP.tile([16, N], bf16, tag="rhs16")
                nc.vector.tensor_mul(rhs16[:], A16[:], m16_sb[:])
                r4s = []
                for b in range(4):
                    r4 = sbP.tile([4, N], bf16, tag=f"r4_{b}")
                    eng = nc.sync if b % 2 == 0 else nc.scalar
                    eng.dma_start(r4[:, 0:2048], rhs16[4 * b:4 * b + 4, 0:2048])
                    eng.dma_start(r4[:, 2048:N], rhs16[4 * b:4 * b + 4, 2048:N])
                    r4s.append(r4)
                dsegs = []
                for b in range(4):
                    dsg = sbP.tile([128, 1], f32, tag=f"dseg{b}")
                    nc.sync.dma_start(dsg[:], dsc[0:1, 128 * b:128 * (b + 1)])
                    dsegs.append(dsg)

                # second PE warm-up batch gated on lhsT4 (bridges the gap
                # between the diag MMs and the first r4-dependent matmul)
                # ---------------- phase C: tile generation ----------------
                with tc.tile_pool(name="psC", bufs=1, space="PSUM") as psC, \
                     tc.tile_pool(name="sbC", bufs=1) as sbC:

                    def ps_copy(dst, src, use_vec):
                        if use_vec:
                            nc.vector.tensor_copy(dst, src)
                        else:
                            nc.scalar.copy(dst, src)

                    # PE warm-up during the post-AR DVE chain (HAM K=8/8)
                    wrm = sbC.tile([128, 512], bf16, tag="wrm")
                    nc.vector.memset(wrm[:], 0.125)
                    dum_ps = psC.tile([32, 512], f32, tag="dum")
                    for _ in range(20):
                        nc.tensor.matmul(dum_ps[:], u_bf[:, 0:32],
                                         wrm[:], start=True, stop=True)

                    dum2_ps = psC.tile([128, 512], f32, tag="dum2")
                    for _ in range(14):
                        nc.tensor.matmul(dum2_ps[:], lhsT4[0:2, 0:128],
                                         wrm[0:2, 0:512], start=True, stop=True)

                    for b in range(4):
                        r4 = r4s[b]
                        lT = lhsT4[:, 128 * b:128 * (b + 1)]
                        stg = sbC.tile([128, N], f32, tag="stg", bufs=2)
                        for h in range(4):
                            mm_ps = psC.tile([128, 1024], f32, tag="chunk", bufs=2)
                            for q in range(2):
                                c0 = 1024 * h + 512 * q
                                nc.tensor.matmul(mm_ps[:, 512 * q:512 * (q + 1)],
                                                 lT, r4[:, c0:c0 + 512],
                                                 start=True, stop=True)
                            ps_copy(stg[:, 1024 * h:1024 * (h + 1)], mm_ps[:],
                                    use_vec=(h % 2 == 1))
                        # diagonal block overwrites the masked hole
                        Tl_ps = psC.tile([128, 128], f32, tag="Tl", bufs=1)
                        nc.tensor.matmul(Tl_ps[:], lhsT4[0:2, 128 * b:128 * (b + 1)],
                                         loc4[0:2, 128 * b:128 * (b + 1)],
                                         start=True, stop=True)
                        Tu_ps = psC.tile([128, 128], f32, tag="Tu", bufs=1)
                        nc.tensor.matmul(Tu_ps[:], loc4[0:2, 128 * b:128 * (b + 1)],
                                         lhsT4[0:2, 128 * b:128 * (b + 1)],
                                         start=True, stop=True)
                        t1 = sbC.tile([128, 128], f32, tag="t1", bufs=2)
                        nc.vector.tensor_mul(t1[:], Tl_ps[:], tril_sb[:])
                        t2 = sbC.tile([128, 128], f32, tag="t2", bufs=2)
                        nc.vector.tensor_mul(t2[:], Tu_ps[:], triu_sb[:])
                        t3 = sbC.tile([128, 128], f32, tag="t3", bufs=2)
                        nc.vector.tensor_scalar(t3[:], diag_sb[:], dsegs[b][:],
                                                None, op0=OP.mult)
                        nc.vector.tensor_add(t1[:], t1[:], t2[:])
                        nc.vector.tensor_add(stg[:, ds(off_loc + 128 * b, 128)],
                                             t1[:], t3[:])
                        nc.sync.dma_start(
                            cov_o[128 * b:128 * (b + 1), 0:2048], stg[:, 0:2048])
                        nc.scalar.dma_start(
                            cov_o[128 * b:128 * (b + 1), 2048:4096],
                            stg[:, 2048:4096])

    nc.compile()
    return nc


def _host_shards(x, mu_kernel, mu_bias, cov_kernel, cov_bias, var_kernel,
                 var_bias, rho_kernel):
    import ml_dtypes
    f = np.float32
    bf = ml_dtypes.bfloat16
    covT = cov_kernel.T.astype(f, copy=False)            # [4096, 2048]
    xT = x.T.astype(f, copy=False)
    w2 = np.stack([rho_kernel[:CDIM], rho_kernel[CDIM:]], axis=1).astype(f)
    # partition-major packs: [j*128+p, c] -> [p, j*C + c]
    w2_pk = np.ascontiguousarray(
        w2.reshape(32, 128, 2).transpose(1, 0, 2).reshape(128, 64).astype(bf))
    cb_pk = np.ascontiguousarray(
        (cov_bias.reshape(32, 128, 1).astype(f) / NCORES)
        .transpose(1, 0, 2).reshape(128, 32).astype(bf))
    mvk = np.ascontiguousarray(
        np.concatenate([mu_kernel.reshape(IDIM, 1), var_kernel.reshape(IDIM, 1)],
                       axis=1).astype(bf))
    iota1 = (np.arange(N, dtype=f) + 1.0).reshape(1, N)
    lstrict = np.ascontiguousarray(np.triu(np.ones((128, 128), f), 1))
    trilS = np.ascontiguousarray(np.tril(np.ones((128, 128), f), -1))
    triuS = np.ascontiguousarray(np.triu(np.ones((128, 128), f), 1))
    diagI = np.ascontiguousarray(np.eye(128, dtype=f))
    mub = np.ascontiguousarray(mu_bias.reshape(1, 1).astype(f) / NCORES)
    vab = np.ascontiguousarray(var_bias.reshape(1, 1).astype(f) / NCORES)

    l = np.arange(N)
    in_maps = []
    for k in range(NCORES):
        m16 = np.zeros((16, N), f)
        for b in range(4):
            r0 = RSH * k + 128 * b
            lo = (l < r0).astype(f)
            up = (l >= r0 + 128).astype(f)
            m16[4 * b + 0] = lo
            m16[4 * b + 1] = lo
            m16[4 * b + 2] = up
            m16[4 * b + 3] = up
        covc = covT[:, ISH * k:ISH * (k + 1)]            # [4096, 256]
        cov_pk = np.ascontiguousarray(
            covc.reshape(32, 128, ISH).transpose(1, 0, 2)
            .reshape(128, 32 * ISH).astype(bf))
        in_maps.append({
            "xT": np.ascontiguousarray(xT[ISH * k:ISH * (k + 1), :].astype(bf)),
            "covT": cov_pk,
            "w2": w2_pk, "cbias": cb_pk,
            "mvk": np.ascontiguousarray(mvk[ISH * k:ISH * (k + 1), :]),
            "mubias": mub, "varbias": vab,
            "iota1": iota1, "lstrict": lstrict, "trilS": trilS,
            "triuS": triuS, "diagI": diagI,
            "m16": m16.astype(bf),
        })
    return in_maps


def kernel(x, mu_kernel, mu_bias, cov_kernel, cov_bias, var_kernel, var_bias,
           rho_kernel, _trace=False):
    _ensure_ntff_hook()
    from concourse import bass_utils

    in_maps = _host_shards(np.asarray(x), np.asarray(mu_kernel),
                           np.asarray(mu_bias), np.asarray(cov_kernel),
                           np.asarray(cov_bias), np.asarray(var_kernel),
                           np.asarray(var_bias), np.asarray(rho_kernel))
    if "nc" not in _BUILT:
        _BUILT["nc"] = _build_nc()
    nc = _BUILT["nc"]

    res = bass_utils.run_bass_kernel_spmd(nc, in_maps, core_ids=list(range(NCORES)),
                                          trace=_trace)
    outs = res.results
    out_cov = np.concatenate([outs[k]["out_cov"] for k in range(NCORES)],
                             axis=0).astype(np.float32, copy=False)
    out_mu = outs[0]["out_mu"].astype(np.float32, copy=False)
    if _trace:
        return (out_mu, out_cov), res
    return out_mu, out_cov
